# revision 14
# baseline (speedup 1.0000x reference)
"""Mamba2 block fused kernel for 8 trn2 NeuronCores.

Sharding: 8 cores = 4 batches x 2 sequence halves (2048 tokens each).
Whole block runs on-device in one NEFF: phase A (LN1 + in_proj(xBC,dt) +
causal conv + chunked SSM state recurrence) -> pairwise AllReduce hands the
SSM state across the sequence split -> phase B1 (intra-chunk scan via
tensor_tensor_scan decay matrices, gated RMSNorm, out_proj, residual, LN2)
-> phase B2 (MLP + residual).  Matmuls run bf16 with fp32 PSUM; decay
matrices are built multiplicatively (scan of dA products) so no large-log
cancellation ever enters bf16.
"""

import numpy as np

D_MODEL = 1024
D_INNER = 2048
D_STATE = 64          # N
D_CONV = 4
HEADDIM = 64          # P
NHEADS = 32           # H
CONV_DIM = D_INNER + 2 * D_STATE                # 2176
D_IN_PROJ = 2 * D_INNER + 2 * D_STATE + NHEADS  # 4256
D_FF = 4096
EPS = 1e-5

NCORES = 8
KD = D_MODEL // 128     # 8
ET_XBC = 18             # e-tiles 16..33 (xBC 17 tiles + dt tile)
ET_Z = 16
CT = 17                 # xBC channel tiles
FT = D_FF // 128        # 32
DT = D_MODEL // 128     # 8

_CACHE = {}


def _bf():
    import ml_dtypes
    return ml_dtypes.bfloat16


# ---------------------------------------------------------------------------
# device program
# ---------------------------------------------------------------------------

def build(tpc, sim=False):
    import concourse.bacc as bacc
    import concourse.tile as tile
    import concourse.mybir as mybir

    f32 = mybir.dt.float32
    bf16 = mybir.dt.bfloat16
    AF = mybir.ActivationFunctionType
    ALU = mybir.AluOpType

    NG = tpc // 512
    NCH = tpc // 128

    nc = bacc.Bacc("TRN2", target_bir_lowering=False)

    # ---- external inputs -------------------------------------------------
    xin = nc.dram_tensor("xin", [tpc, D_MODEL], f32, kind="ExternalInput")
    xhalo = nc.dram_tensor("xhalo", [3, D_MODEL], f32, kind="ExternalInput")
    cmask = nc.dram_tensor("cmask", [128, 2], f32, kind="ExternalInput")
    lnw = nc.dram_tensor("lnw", [128, 4, D_MODEL], f32, kind="ExternalInput")
    wxbc_d = nc.dram_tensor("wxbc", [128, KD, ET_XBC, 128], bf16,
                            kind="ExternalInput")
    wz_d = nc.dram_tensor("wz", [128, KD, ET_Z, 128], bf16,
                          kind="ExternalInput")
    wout_d = nc.dram_tensor("wout", [128, 16, DT, 128], bf16,
                            kind="ExternalInput")
    wfc_d = nc.dram_tensor("wfc", [128, KD, FT, 128], bf16,
                           kind="ExternalInput")
    wpj_d = nc.dram_tensor("wpj", [128, FT, DT, 128], bf16,
                           kind="ExternalInput")
    convw = nc.dram_tensor("convw", [128, CT, D_CONV], f32,
                           kind="ExternalInput")
    convb = nc.dram_tensor("convb", [128, CT], f32, kind="ExternalInput")
    dtb = nc.dram_tensor("dtb", [32, 1], f32, kind="ExternalInput")
    negA = nc.dram_tensor("negA", [32, 1], f32, kind="ExternalInput")
    dcol = nc.dram_tensor("dcol", [128, 16], f32, kind="ExternalInput")
    nwcol = nc.dram_tensor("nwcol", [128, 16], f32, kind="ExternalInput")
    fcb_d = nc.dram_tensor("fcb", [128, FT], f32, kind="ExternalInput")
    pjb_d = nc.dram_tensor("pjb", [128, DT], f32, kind="ExternalInput")
    d32_d = nc.dram_tensor("d32", [32, 32], bf16, kind="ExternalInput")
    idrep_d = nc.dram_tensor("idrep", [128, 16, 128], bf16,
                             kind="ExternalInput")
    identb_d = nc.dram_tensor("identb", [128, 128], bf16, kind="ExternalInput")
    identf_d = nc.dram_tensor("identf", [128, 128], f32, kind="ExternalInput")
    if sim:
        hin_dbg = nc.dram_tensor("hin_dbg", [64, D_INNER], f32,
                                 kind="ExternalInput")

    # ---- dram scratch ----------------------------------------------------
    sp_xnt = nc.dram_tensor("sp_xnt", [128, KD, tpc], bf16)
    sp_xbc = nc.dram_tensor("sp_xbc", [128, CT, tpc], bf16)
    sp_dtxt = nc.dram_tensor("sp_dtxt", [128, NCH, D_INNER], bf16)
    sp_h0 = nc.dram_tensor("sp_h0", [64, NCH, D_INNER], bf16)
    sp_cl = nc.dram_tensor("sp_cl", [32, tpc], f32)
    sp_tdl = nc.dram_tensor("sp_tdl", [32, NCH + 1], f32)
    sp_dae = nc.dram_tensor("sp_dae", [32, tpc], bf16)
    sp_u = nc.dram_tensor("sp_u", [128, NCH, D_MODEL], f32)
    sp_xn2 = nc.dram_tensor("sp_xn2", [128, NCH, D_MODEL], bf16)
    inb = nc.dram_tensor("inb", [64, D_INNER], f32)
    outb = nc.dram_tensor("outb", [64, D_INNER], f32)

    out_d = nc.dram_tensor("out", [tpc, D_MODEL], f32, kind="ExternalOutput")

    def _ln_tile(pool, xt, wrow, brow, epsc, p, tag):
        st = pool.tile([128, 2, 6], f32, tag=f"lnst{tag}")
        mv = pool.tile([128, 2], f32, tag=f"lnmv{tag}")
        xr = xt[:p].rearrange("p (n q) -> p n q", n=2)
        for i in range(2):
            nc.vector.bn_stats(out=st[:p, i, :], in_=xr[:, i, :])
        nc.vector.bn_aggr(out=mv[:p], in_=st[:p])
        nc.scalar.activation(out=mv[:p, 1:2], in_=mv[:p, 1:2], func=AF.Sqrt,
                             bias=epsc[:p], scale=1.0)
        nc.vector.reciprocal(out=mv[:p, 1:2], in_=mv[:p, 1:2])
        t1 = pool.tile([128, D_MODEL], f32, tag=f"lnt1{tag}")
        nc.vector.tensor_scalar(out=t1[:p], in0=xt[:p], scalar1=mv[:p, 0:1],
                                scalar2=mv[:p, 1:2], op0=ALU.subtract,
                                op1=ALU.mult)
        t2 = pool.tile([128, D_MODEL], bf16, tag=f"lnt2{tag}")
        nc.vector.tensor_tensor(out=t2[:p], in0=t1[:p], in1=wrow[:p],
                                op=ALU.mult)
        nc.vector.tensor_add(out=t2[:p], in0=t2[:p], in1=brow[:p])
        return t2

    def act_silu(pool, out_ap, in_ap, bias, tag):
        if not sim:
            nc.scalar.activation(out_ap, in_ap, AF.Silu, bias=bias, scale=1.0)
            return
        shp = list(out_ap.shape)
        xb = pool.tile(shp, f32, tag=f"silux{tag}")
        nc.scalar.activation(xb[:], in_ap, AF.Identity, bias=bias, scale=1.0)
        sg = pool.tile(shp, f32, tag=f"silus{tag}")
        nc.scalar.activation(sg[:], xb[:], AF.Sigmoid)
        nc.vector.tensor_tensor(out=out_ap, in0=xb[:], in1=sg[:], op=ALU.mult)

    def act_gelu(pool, out_ap, in_ap, bias, tag):
        if not sim:
            nc.scalar.activation(out_ap, in_ap, AF.Gelu, bias=bias, scale=1.0)
            return
        shp = list(out_ap.shape)
        v = pool.tile(shp, f32, tag=f"gelv{tag}")
        nc.scalar.activation(v[:], in_ap, AF.Identity, bias=bias, scale=1.0)
        v3 = pool.tile(shp, f32, tag=f"gel3{tag}")
        nc.scalar.activation(v3[:], v[:], AF.Square)
        nc.vector.tensor_tensor(out=v3[:], in0=v3[:], in1=v[:], op=ALU.mult)
        nc.vector.tensor_scalar(out=v3[:], in0=v3[:],
                                scalar1=0.7978845608 * 0.044715, scalar2=0.0,
                                op0=ALU.mult, op1=ALU.add)
        nc.vector.scalar_tensor_tensor(out=v3[:], in0=v[:],
                                       scalar=0.7978845608, in1=v3[:],
                                       op0=ALU.mult, op1=ALU.add)
        nc.scalar.activation(v3[:], v3[:], AF.Tanh)
        nc.vector.tensor_scalar(out=v3[:], in0=v3[:], scalar1=1.0,
                                scalar2=0.5, op0=ALU.add, op1=ALU.mult)
        nc.vector.tensor_tensor(out=out_ap, in0=v[:], in1=v3[:], op=ALU.mult)

    # =====================================================================
    # PHASE A
    # =====================================================================
    with tile.TileContext(nc) as tc:
        with (
            tc.tile_pool(name="consts", bufs=1) as cst,
            tc.tile_pool(name="wa", bufs=1) as wa,
            tc.tile_pool(name="big1", bufs=1) as bg,
            tc.tile_pool(name="xio", bufs=3) as xio,
            tc.tile_pool(name="sm", bufs=2) as sm,
            tc.tile_pool(name="work", bufs=2) as wk,
            tc.tile_pool(name="stp", bufs=1) as stp,
            tc.tile_pool(name="psq", bufs=2, space="PSUM") as ppq,
            tc.tile_pool(name="pssm", bufs=2, space="PSUM") as pps,
            tc.tile_pool(name="psS", bufs=1, space="PSUM") as ppS,
        ):
            identb = cst.tile([128, 128], bf16)
            nc.sync.dma_start(identb[:], identb_d[:])
            identf = cst.tile([128, 128], f32)
            nc.sync.dma_start(identf[:], identf_d[:])
            epsc = cst.tile([128, 1], f32)
            nc.vector.memset(epsc[:], EPS)
            ln1w = cst.tile([128, D_MODEL], f32, tag="ln1w")
            nc.sync.dma_start(ln1w[:], lnw[:, 0, :])
            ln1b = cst.tile([128, D_MODEL], f32, tag="ln1b")
            nc.sync.dma_start(ln1b[:], lnw[:, 1, :])
            cw = cst.tile([128, CT, D_CONV], f32)
            nc.sync.dma_start(cw[:], convw[:])
            cb = cst.tile([128, CT], f32)
            nc.sync.dma_start(cb[:], convb[:])
            dtbt = cst.tile([32, 1], f32)
            nc.sync.dma_start(dtbt[:], dtb[:])
            negAt = cst.tile([32, 1], f32)
            nc.sync.dma_start(negAt[:], negA[:])
            ones1_64 = cst.tile([1, 64], f32)
            nc.vector.memset(ones1_64[:], 1.0)
            cmt = cst.tile([128, 2], f32)
            nc.sync.dma_start(cmt[:], cmask[:])

            wxbc = wa.tile([128, KD, ET_XBC, 128], bf16)
            nc.sync.dma_start(wxbc[:], wxbc_d[:])

            hst = stp.tile([64, D_INNER], f32)
            nc.vector.memset(hst[:], 0.0)
            tdl = stp.tile([32, NCH + 1], f32)
            nc.vector.memset(tdl[:], 0.0)

            # halo xBC (tokens -3..-1), raw (pre-silu)
            xh = xio.tile([3, D_MODEL], f32, tag="xh")
            nc.sync.dma_start(xh[:], xhalo[:])
            xhn = _ln_tile(wk, xh, ln1w, ln1b, epsc, 3, "a")
            xhnt = wk.tile([128, KD, 3], bf16, tag="xhnt")
            for k in range(KD):
                pst = pps.tile([128, 128], bf16, tag="psA")
                nc.tensor.transpose(pst[:, 0:3],
                                    xhn[0:3, k * 128:(k + 1) * 128],
                                    identb[0:3, 0:3])
                nc.scalar.copy(xhnt[:, k, :], pst[:, 0:3])
            halo = bg.tile([3, CONV_DIM], bf16, tag="halo")
            for e in range(CT):
                psh = pps.tile([3, 128], f32, tag="psA")
                for k in range(KD):
                    nc.tensor.matmul(psh[:], xhnt[:, k, :], wxbc[:, k, e, :],
                                     start=(k == 0), stop=(k == KD - 1))
                nc.scalar.copy(halo[:, e * 128:(e + 1) * 128], psh[:])
            tail = sm.tile([128, CT, 3], bf16, tag="tail")
            for e in range(CT):
                pst = pps.tile([128, 128], bf16, tag="psA")
                nc.tensor.transpose(pst[:, 0:3],
                                    halo[0:3, e * 128:(e + 1) * 128],
                                    identb[0:3, 0:3])
                nc.scalar.copy(tail[:, e, :], pst[:, 0:3])
            for g in range(NG):
                gsl = slice(g * 512, (g + 1) * 512)
                # ---- LN1 + transpose to xnt ----
                xnt = sm.tile([128, KD, 512], bf16, tag="xnt")
                for j in range(4):
                    xt = xio.tile([128, D_MODEL], f32, tag="xt")
                    nc.sync.dma_start(
                        xt[:],
                        xin[g * 512 + j * 128: g * 512 + (j + 1) * 128, :])
                    xn = _ln_tile(wk, xt, ln1w, ln1b, epsc, 128, "a")
                    for k in range(KD):
                        pst = pps.tile([128, 128], bf16, tag="psA")
                        nc.tensor.transpose(
                            pst[:], xn[:, k * 128:(k + 1) * 128], identb[:])
                        nc.scalar.copy(xnt[:, k, j * 128:(j + 1) * 128],
                                       pst[:])
                nc.sync.dma_start(sp_xnt[:, :, gsl], xnt[:])

                # ---- in_proj xBC + dt ----
                xrg = bg.tile([128, CT, 515], bf16, tag="xrg")
                dt_f = wk.tile([32, 512], f32, tag="dtf")
                for e in range(ET_XBC):
                    psq = ppq.tile([128, 512], f32, tag="psq")
                    for k in range(KD):
                        nc.tensor.matmul(psq[:], wxbc[:, k, e, :],
                                         xnt[:, k, :],
                                         start=(k == 0), stop=(k == KD - 1))
                    if e < CT:
                        nc.scalar.copy(xrg[:, e, 3:515], psq[:])
                        nc.vector.tensor_copy(xrg[:, e, 0:3], tail[:, e, :])
                    else:
                        t_e = wk.tile([32, 512], f32, tag="cacc")
                        nc.scalar.activation(t_e[:], psq[0:32, :], AF.Exp,
                                             bias=dtbt[:], scale=1.0)
                        nc.scalar.activation(dt_f[:], t_e[:], AF.Ln,
                                             bias=1.0, scale=1.0)

                # ---- conv + silu ----
                xsg = bg.tile([128, CT, 512], bf16, tag="xsg")
                for ct in range(CT):
                    acc = wk.tile([128, 512], f32, tag="cacc")
                    nc.vector.tensor_scalar(
                        out=acc[:], in0=xrg[:, ct, 0:512],
                        scalar1=cw[:, ct, 0:1], scalar2=0.0,
                        op0=ALU.mult, op1=ALU.add)
                    for k in range(1, D_CONV):
                        nc.vector.scalar_tensor_tensor(
                            out=acc[:], in0=xrg[:, ct, k:k + 512],
                            scalar=cw[:, ct, k:k + 1], in1=acc[:],
                            op0=ALU.mult, op1=ALU.add)
                    act_silu(wk, xsg[:, ct, :], acc[:], cb[:, ct:ct + 1],
                             "cv")
                nc.sync.dma_start(sp_xbc[:, :, gsl], xsg[:])
                tail = sm.tile([128, CT, 3], bf16, tag="tail")
                for e in range(CT):
                    nc.vector.tensor_copy(tail[:, e, :], xrg[:, e, 512:515])

                # ---- per chunk: dt maths + S matmul + state ----
                clg = wk.tile([32, 512], f32, tag="clg")
                daeg = wk.tile([32, 512], bf16, tag="daeg")
                for c in range(4):
                    gc = g * 4 + c
                    sl = slice(c * 128, (c + 1) * 128)
                    dal = wk.tile([32, 128], f32, tag="dal")
                    nc.vector.tensor_scalar(
                        out=dal[:], in0=dt_f[:, sl], scalar1=negAt[:],
                        scalar2=0.0, op0=ALU.mult, op1=ALU.add)
                    nc.vector.tensor_tensor_scan(
                        out=clg[:, sl], data0=dal[:], data1=dal[:],
                        initial=0.0, op0=ALU.add, op1=ALU.bypass)
                    nc.scalar.activation(daeg[:, sl], dal[:], AF.Exp)
                    # dA col 0 of each chunk zeroed (M-scan reset injection)
                    nc.vector.memset(daeg[:, c * 128:c * 128 + 1], 0.0)
                    # wS = exp(cl_last - cl)
                    sfx = wk.tile([32, 128], f32, tag="sfx")
                    nc.vector.tensor_scalar(
                        out=sfx[:], in0=clg[:, sl],
                        scalar1=clg[:, c * 128 + 127:c * 128 + 128],
                        scalar2=-1.0, op0=ALU.subtract, op1=ALU.mult)
                    nc.scalar.activation(sfx[:], sfx[:], AF.Exp)
                    pswt = pps.tile([128, 128], f32, tag="psA")
                    nc.tensor.transpose(pswt[:, 0:32], sfx[:], identf[0:32, 0:32])
                    wst = wk.tile([128, 32], f32, tag="wst")
                    nc.scalar.copy(wst[:], pswt[:, 0:32])
                    psdt = pps.tile([128, 128], f32, tag="psA")
                    nc.tensor.transpose(psdt[:, 0:32], dt_f[:, sl], identf[0:32, 0:32])
                    dtt = wk.tile([128, 32], f32, tag="dtt")
                    nc.scalar.copy(dtt[:], psdt[:, 0:32])
                    # xsT -> dtxT fused on psum evacuation
                    dtxt = wk.tile([128, 32, 64], bf16, tag="dtxt")
                    for ct in range(16):
                        pst = pps.tile([128, 128], bf16, tag="psA")
                        nc.tensor.transpose(pst[:], xsg[:, ct, sl], identb[:])
                        nc.vector.tensor_tensor(
                            out=dtxt[:, 2 * ct:2 * ct + 2, :],
                            in0=pst[:].rearrange("p (h q) -> p h q", h=2),
                            in1=dtt[:, 2 * ct:2 * ct + 2].unsqueeze(2)
                            .broadcast_to([128, 2, 64]),
                            op=ALU.mult)
                    nc.sync.dma_start(
                        sp_dtxt[:, gc, :],
                        dtxt[:].rearrange("p a b -> p (a b)"))
                    wdt = wk.tile([128, 32, 64], bf16, tag="wdt")
                    nc.vector.tensor_tensor(
                        out=wdt[:], in0=dtxt[:],
                        in1=wst[:].unsqueeze(2).broadcast_to([128, 32, 64]),
                        op=ALU.mult)
                    # B_st
                    psb = pps.tile([128, 128], bf16, tag="psA")
                    nc.tensor.transpose(psb[:, 0:64], xsg[0:64, 16, sl],
                                        identb[0:64, 0:64])
                    bst = wk.tile([128, 64], bf16, tag="bst")
                    nc.scalar.copy(bst[:], psb[:, 0:64])
                    # S matmuls
                    psS = ppS.tile([64, D_INNER], f32, tag="psS")
                    for h in range(NHEADS):
                        nc.tensor.matmul(
                            psS[:, h * 64:(h + 1) * 64], bst[:],
                            wdt[:, h, :], start=True, stop=True)
                    # Dec = exp(cl_last) broadcast [64, 32]
                    psd = pps.tile([128, 128], f32, tag="psA")
                    nc.tensor.transpose(
                        psd[0:1, 0:32],
                        clg[:, c * 128 + 127:c * 128 + 128],
                        identf[0:32, 0:32])
                    decr = wk.tile([1, 32], f32, tag="decr")
                    nc.scalar.copy(decr[:], psd[0:1, 0:32])
                    psd2 = pps.tile([128, 128], f32, tag="psA")
                    nc.tensor.matmul(psd2[0:64, 0:32], ones1_64[:], decr[:],
                                     start=True, stop=True)
                    dec = wk.tile([64, 32], f32, tag="dec")
                    nc.scalar.activation(dec[:], psd2[0:64, 0:32], AF.Exp)
                    # tdl
                    nc.vector.tensor_add(
                        out=tdl[:, gc + 1:gc + 2], in0=tdl[:, gc:gc + 1],
                        in1=clg[:, c * 128 + 127:c * 128 + 128])
                    # store h0 then update state
                    h0c = wk.tile([64, D_INNER], bf16, tag="h0c")
                    nc.vector.tensor_copy(h0c[:], hst[:])
                    nc.sync.dma_start(sp_h0[:, gc, :], h0c[:])
                    nc.vector.tensor_tensor(
                        out=hst[:].rearrange("n (h q) -> n h q", h=32),
                        in0=hst[:].rearrange("n (h q) -> n h q", h=32),
                        in1=dec[:].unsqueeze(2).broadcast_to([64, 32, 64]),
                        op=ALU.mult)
                    nc.vector.tensor_add(out=hst[:], in0=hst[:], in1=psS[:])
                nc.sync.dma_start(sp_cl[:, gsl], clg[:])
                nc.sync.dma_start(sp_dae[:, gsl], daeg[:])

            nc.sync.dma_start(sp_tdl[:], tdl[:])
            hm = wk.tile([64, D_INNER], f32, tag="h0c")
            nc.vector.tensor_scalar(out=hm[:], in0=hst[:],
                                    scalar1=cmt[0:64, 0:1], scalar2=0.0,
                                    op0=ALU.mult, op1=ALU.add)
            nc.sync.dma_start(inb[:], hm[:])

    # =====================================================================
    # collective (hw only)
    # =====================================================================
    if not sim:
        with nc.semaphore("cc_sem") as cc_sem:
            nc.gpsimd.collective_compute(
                "AllReduce", mybir.AluOpType.add,
                replica_groups=[[0, 1], [2, 3], [4, 5], [6, 7]],
                ins=[inb.ap().opt()], outs=[outb.ap().opt()],
            ).then_inc(cc_sem)
            nc.gpsimd.wait_ge(cc_sem, 1)
            nc.all_engine_barrier()

    # =====================================================================
    # PHASE B1
    # =====================================================================
    with tile.TileContext(nc) as tc:
        with (
            tc.tile_pool(name="cst2", bufs=1) as cst,
            tc.tile_pool(name="wb", bufs=1) as wb,
            tc.tile_pool(name="big2", bufs=1) as bg,
            tc.tile_pool(name="str2", bufs=4) as st4,
            tc.tile_pool(name="wk2", bufs=2) as wk,
            tc.tile_pool(name="one2", bufs=1) as on1,
            tc.tile_pool(name="wk1", bufs=1) as wk1,
            tc.tile_pool(name="ps2", bufs=2, space="PSUM") as pp,
            tc.tile_pool(name="psbig", bufs=1, space="PSUM") as ppb,
            tc.tile_pool(name="psy", bufs=2, space="PSUM") as ppy,
        ):
            identb = cst.tile([128, 128], bf16)
            nc.sync.dma_start(identb[:], identb_d[:])
            identf = cst.tile([128, 128], f32)
            nc.sync.dma_start(identf[:], identf_d[:])
            epsc = cst.tile([128, 1], f32)
            nc.vector.memset(epsc[:], EPS)
            eps1 = cst.tile([1, 1], f32)
            nc.vector.memset(eps1[:], EPS)
            ln2w = cst.tile([128, D_MODEL], f32, tag="ln2w")
            nc.sync.dma_start(ln2w[:], lnw[:, 2, :])
            ln2b = cst.tile([128, D_MODEL], f32, tag="ln2b")
            nc.sync.dma_start(ln2b[:], lnw[:, 3, :])
            cmt = cst.tile([128, 2], f32)
            nc.sync.dma_start(cmt[:], cmask[:])
            dcolt = cst.tile([128, 16], f32)
            nc.sync.dma_start(dcolt[:], dcol[:])
            nwt = cst.tile([128, 16], f32)
            nc.sync.dma_start(nwt[:], nwcol[:])
            d32 = cst.tile([32, 32], bf16)
            nc.sync.dma_start(d32[:], d32_d[:])
            idrep = cst.tile([128, 16, 128], bf16)
            nc.sync.dma_start(idrep[:], idrep_d[:])
            ones32_128 = cst.tile([32, 128], bf16)
            nc.vector.memset(ones32_128[:], 1.0)
            ones32_64 = cst.tile([32, 64], bf16)
            nc.vector.memset(ones32_64[:], 1.0)
            ones128_1 = cst.tile([128, 1], bf16)
            nc.vector.memset(ones128_1[:], 1.0)
            ones1_128f = cst.tile([1, 128], f32)
            nc.vector.memset(ones1_128f[:], 1.0)
            ones1_64 = cst.tile([1, 64], f32)
            nc.vector.memset(ones1_64[:], 1.0)

            wout = wb.tile([128, 16, DT, 128], bf16)
            nc.sync.dma_start(wout[:], wout_d[:])

            cl_all = wb.tile([32, tpc], f32, tag="clall")
            nc.sync.dma_start(cl_all[:], sp_cl[:])
            dae_all = wb.tile([32, tpc], bf16, tag="daeall")
            nc.sync.dma_start(dae_all[:], sp_dae[:])
            tdl_t = wb.tile([32, NCH + 1], f32, tag="tdlt")
            nc.sync.dma_start(tdl_t[:], sp_tdl[:])
            hin = wb.tile([64, D_INNER], f32, tag="hin")
            if sim:
                nc.sync.dma_start(hin[:], hin_dbg[:])
            else:
                nc.sync.dma_start(hin[:], outb[:])
            nc.vector.tensor_scalar(out=hin[:], in0=hin[:],
                                    scalar1=cmt[0:64, 1:2], scalar2=0.0,
                                    op0=ALU.mult, op1=ALU.add)

            for g in range(NG):
                gsl = slice(g * 512, (g + 1) * 512)
                xsg = bg.tile([128, CT, 512], bf16, tag="xsg2")
                nc.sync.dma_start(xsg[:], sp_xbc[:, :, gsl])
                xnt = bg.tile([128, KD, 512], bf16, tag="xnt2")
                nc.sync.dma_start(xnt[:], sp_xnt[:, :, gsl])
                ctg = bg.tile([64, 512], bf16, tag="ctg")
                nc.vector.tensor_copy(ctg[:], xsg[64:128, 16, :])

                ynt = bg.tile([128, 16, 512], bf16, tag="ynt")
                ug = bg.tile([128, 4, D_MODEL], f32, tag="ug")
                for c in range(4):
                    gc = g * 4 + c
                    sl = slice(c * 128, (c + 1) * 128)
                    gcsl = slice(gc * 128, (gc + 1) * 128)
                    # h0 true = h0_local + Lambda * h_in
                    pslm = pp.tile([128, 512], f32, tag="pssm")
                    nc.tensor.transpose(pslm[0:1, 0:32],
                                        tdl_t[:, gc:gc + 1],
                                        identf[0:32, 0:32])
                    lamr = wk.tile([1, 32], f32, tag="lamr")
                    nc.scalar.copy(lamr[:], pslm[0:1, 0:32])
                    pslm2 = pp.tile([128, 512], f32, tag="pssm")
                    nc.tensor.matmul(pslm2[0:64, 0:32], ones1_64[:], lamr[:],
                                     start=True, stop=True)
                    lam = wk.tile([64, 32], f32, tag="lam")
                    nc.scalar.activation(lam[:], pslm2[0:64, 0:32], AF.Exp)
                    h0r = wk1.tile([64, 32, 64], bf16, tag="h0r")
                    nc.sync.dma_start(
                        h0r[:].rearrange("n a b -> n (a b)"), sp_h0[:, gc, :])
                    h0t = bg.tile([64, 32, 64], bf16, tag="h0t")
                    nc.vector.tensor_tensor(
                        out=h0t[:],
                        in0=hin[:].rearrange("n (h q) -> n h q", h=32),
                        in1=lam[:].unsqueeze(2).broadcast_to([64, 32, 64]),
                        op=ALU.mult)
                    nc.vector.tensor_add(out=h0t[:], in0=h0t[:], in1=h0r[:])
                    # G^T
                    psG = pp.tile([128, 512], f32, tag="pssm")
                    nc.tensor.matmul(psG[:, 0:128], xsg[0:64, 16, sl],
                                     ctg[:, sl], start=True, stop=True)
                    dtxt = wk.tile([128, 32, 64], bf16, tag="dtxt2")
                    nc.sync.dma_start(
                        dtxt[:].rearrange("p a b -> p (a b)"),
                        sp_dtxt[:, gc, :])
                    w0f = wk.tile([32, 128], f32, tag="w0f")
                    nc.scalar.activation(w0f[:], cl_all[:, gcsl], AF.Exp)
                    ycur = bg.tile([128, 16, 128], f32, tag="ycur")
                    for hf in range(2):
                        hsl = slice(hf * 16, (hf + 1) * 16)
                        bph = wk.tile([32, 16, 128], bf16, tag="bph")
                        nc.vector.tensor_tensor(
                            out=bph[:],
                            in0=d32[:, hsl].unsqueeze(2)
                            .broadcast_to([32, 16, 128]),
                            in1=dae_all[:, gcsl].unsqueeze(1)
                            .broadcast_to([32, 16, 128]),
                            op=ALU.mult)
                        psdab = ppb.tile([128, 2048], f32, tag="psbg")
                        for q in range(4):
                            nc.tensor.matmul(
                                psdab[:, q * 512:(q + 1) * 512],
                                ones32_128[:],
                                bph[:].rearrange("p a b -> p (a b)")
                                [:, q * 512:(q + 1) * 512],
                                start=True, stop=True)
                        mh = wk.tile([128, 16, 128], bf16, tag="mh")
                        nc.vector.tensor_tensor_scan(
                            out=mh[:].rearrange("p a b -> p (a b)"),
                            data0=psdab[:],
                            data1=idrep[:].rearrange("p a b -> p (a b)"),
                            initial=0.0, op0=ALU.mult, op1=ALU.add)
                        nc.vector.tensor_tensor(
                            out=mh[:],
                            in0=mh[:],
                            in1=psG[:, 0:128].unsqueeze(1)
                            .broadcast_to([128, 16, 128]),
                            op=ALU.mult)
                        wph = wk.tile([32, 16, 128], bf16, tag="bph")
                        nc.vector.tensor_tensor(
                            out=wph[:],
                            in0=d32[:, hsl].unsqueeze(2)
                            .broadcast_to([32, 16, 128]),
                            in1=w0f[:].unsqueeze(1).broadcast_to([32, 16, 128]),
                            op=ALU.mult)
                        psw0 = ppb.tile([128, 2048], f32, tag="psbg")
                        for q in range(4):
                            nc.tensor.matmul(
                                psw0[0:64, q * 512:(q + 1) * 512],
                                ones32_64[:],
                                wph[:].rearrange("p a b -> p (a b)")
                                [:, q * 512:(q + 1) * 512],
                                start=True, stop=True)
                        cwt = wk.tile([64, 16, 128], bf16, tag="cwt")
                        nc.vector.tensor_tensor(
                            out=cwt[:],
                            in0=psw0[0:64].rearrange("n (a b) -> n a b", a=16),
                            in1=ctg[:, sl].unsqueeze(1)
                            .broadcast_to([64, 16, 128]), op=ALU.mult)
                        for hp in range(8):
                            h0i = hf * 16 + hp * 2
                            psyt = ppy.tile([128, 128], f32, tag="psy")
                            for par in range(2):
                                h = h0i + par
                                po = slice(par * 64, (par + 1) * 64)
                                nc.tensor.matmul(
                                    psyt[po, :], dtxt[:, h, :],
                                    mh[:, hp * 2 + par, :],
                                    start=True, stop=False)
                                nc.tensor.matmul(
                                    psyt[po, :], h0t[:, h, :],
                                    cwt[:, hp * 2 + par, :],
                                    start=False, stop=True)
                            cti = h0i // 2
                            nc.vector.scalar_tensor_tensor(
                                out=ycur[:, cti, :], in0=xsg[:, cti, sl],
                                scalar=dcolt[:, cti:cti + 1], in1=psyt[:],
                                op0=ALU.mult, op1=ALU.add)
                    # z + gated rmsnorm
                    ysq = wk1.tile([128, 16, 128], bf16, tag="ysq")
                    for e in range(16):
                        wzt = st4.tile([128, KD, 128], bf16, tag="wzt")
                        nc.sync.dma_start(wzt[:], wz_d[:, :, e, :])
                        psz = pp.tile([128, 512], f32, tag="pssm")
                        for k in range(KD):
                            nc.tensor.matmul(psz[:, 0:128], wzt[:, k, :],
                                             xnt[:, k, sl],
                                             start=(k == 0),
                                             stop=(k == KD - 1))
                        zsil = wk.tile([128, 128], bf16, tag="zsil")
                        act_silu(wk, zsil[:], psz[:, 0:128], 0.0, "z")
                        nc.vector.tensor_tensor(
                            out=ycur[:, e, :], in0=ycur[:, e, :],
                            in1=zsil[:], op=ALU.mult)
                        nc.scalar.activation(ysq[:, e, :], ycur[:, e, :],
                                             AF.Square)
                    psss = pp.tile([128, 512], f32, tag="pssm")
                    for e in range(16):
                        nc.tensor.matmul(psss[0:1, 0:128], ones128_1[:],
                                         ysq[:, e, :],
                                         start=(e == 0), stop=(e == 15))
                    rst = wk.tile([1, 128], f32, tag="rst")
                    nc.scalar.activation(rst[:], psss[0:1, 0:128], AF.Ln,
                                         bias=eps1[:], scale=1.0 / D_INNER)
                    nc.scalar.activation(rst[:], rst[:], AF.Exp, scale=-0.5)
                    psr = pp.tile([128, 512], f32, tag="pssm")
                    nc.tensor.matmul(psr[:, 0:128], ones1_128f[:], rst[:],
                                     start=True, stop=True)
                    for e in range(16):
                        nc.vector.scalar_tensor_tensor(
                            out=ynt[:, e, sl], in0=ycur[:, e, :],
                            scalar=nwt[:, e:e + 1], in1=psr[:, 0:128],
                            op0=ALU.mult, op1=ALU.mult)

                # out_proj + residual
                for d in range(DT):
                    psu = pp.tile([128, 512], f32, tag="pssm")
                    for e in range(16):
                        nc.tensor.matmul(psu[:], wout[:, e, d, :],
                                         ynt[:, e, :],
                                         start=(e == 0), stop=(e == 15))
                    mt = wk1.tile([128, 512], bf16, tag="mt")
                    nc.scalar.copy(mt[:], psu[:])
                    for j in range(4):
                        pstr = ppy.tile([128, 128], bf16, tag="psy")
                        nc.tensor.transpose(pstr[:],
                                            mt[:, j * 128:(j + 1) * 128],
                                            identb[:])
                        xt2 = st4.tile([128, 128], f32, tag="xt2")
                        nc.sync.dma_start(
                            xt2[:],
                            xin[g * 512 + j * 128: g * 512 + (j + 1) * 128,
                                d * 128:(d + 1) * 128])
                        nc.vector.tensor_add(
                            out=ug[:, j, d * 128:(d + 1) * 128],
                            in0=xt2[:], in1=pstr[:])
                # LN2 + spills
                for j in range(4):
                    gc = g * 4 + j
                    nc.sync.dma_start(sp_u[:, gc, :], ug[:, j, :])
                    xn2 = _ln_tile(wk1, ug[:, j, :], ln2w, ln2b, epsc, 128,
                                   "b")
                    nc.sync.dma_start(sp_xn2[:, gc, :], xn2[:])

    # =====================================================================
    # PHASE B2: MLP
    # =====================================================================
    with tile.TileContext(nc) as tc:
        with (
            tc.tile_pool(name="cst3", bufs=1) as cst,
            tc.tile_pool(name="wm", bufs=1) as wm,
            tc.tile_pool(name="wk3", bufs=2) as wk,
            tc.tile_pool(name="ht", bufs=1) as htp,
            tc.tile_pool(name="ps3", bufs=2, space="PSUM") as pp,
            tc.tile_pool(name="ps3b", bufs=2, space="PSUM") as pp2,
        ):
            identb = cst.tile([128, 128], bf16)
            nc.sync.dma_start(identb[:], identb_d[:])
            fcbt = cst.tile([128, FT], f32)
            nc.sync.dma_start(fcbt[:], fcb_d[:])
            pjbt = cst.tile([128, DT], f32)
            nc.sync.dma_start(pjbt[:], pjb_d[:])
            wfc = wm.tile([128, KD, FT, 128], bf16, tag="wfc")
            nc.sync.dma_start(wfc[:], wfc_d[:])
            wpj = wm.tile([128, FT, DT, 128], bf16, tag="wpj")
            nc.sync.dma_start(wpj[:], wpj_d[:])

            for g in range(NG):
                xn2t = wk.tile([128, KD, 512], bf16, tag="xn2t")
                for j in range(4):
                    gc = g * 4 + j
                    xn2 = wk.tile([128, D_MODEL], bf16, tag="xn2r")
                    nc.sync.dma_start(xn2[:], sp_xn2[:, gc, :])
                    for k in range(KD):
                        pst = pp2.tile([128, 128], bf16, tag="psT3")
                        nc.tensor.transpose(
                            pst[:], xn2[:, k * 128:(k + 1) * 128], identb[:])
                        nc.scalar.copy(xn2t[:, k, j * 128:(j + 1) * 128],
                                       pst[:])
                hT = htp.tile([128, FT, 512], bf16, tag="hT")
                for f in range(FT):
                    psf = pp.tile([128, 512], f32, tag="psf")
                    for k in range(KD):
                        nc.tensor.matmul(psf[:], wfc[:, k, f, :],
                                         xn2t[:, k, :],
                                         start=(k == 0), stop=(k == KD - 1))
                    act_gelu(wk, hT[:, f, :], psf[:], fcbt[:, f:f + 1],
                             "fc")
                for d in range(DT):
                    pso = pp.tile([128, 512], f32, tag="psf")
                    for f in range(FT):
                        nc.tensor.matmul(pso[:], wpj[:, f, d, :], hT[:, f, :],
                                         start=(f == 0), stop=(f == FT - 1))
                    obf = wk.tile([128, 512], bf16, tag="obf")
                    nc.scalar.activation(obf[:], pso[:], AF.Identity,
                                         bias=pjbt[:, d:d + 1], scale=1.0)
                    for j in range(4):
                        gc = g * 4 + j
                        pstr = pp2.tile([128, 128], bf16, tag="psT3")
                        nc.tensor.transpose(
                            pstr[:], obf[:, j * 128:(j + 1) * 128], identb[:])
                        urd = wk.tile([128, 128], f32, tag="urd")
                        nc.sync.dma_start(
                            urd[:], sp_u[:, gc, d * 128:(d + 1) * 128])
                        fo = wk.tile([128, 128], f32, tag="fo")
                        nc.vector.tensor_add(out=fo[:], in0=urd[:],
                                             in1=pstr[:])
                        nc.sync.dma_start(
                            out_d[g * 512 + j * 128: g * 512 + (j + 1) * 128,
                                  d * 128:(d + 1) * 128], fo[:])

    nc.compile()
    return nc


# ---------------------------------------------------------------------------
# host side
# ---------------------------------------------------------------------------

def _prep_shared(ln1_w, ln1_b, ln2_w, ln2_b, in_proj_w, conv_w, conv_b,
                 dt_bias, A_log, D_param, norm_w, out_proj_w, fc_w, fc_b,
                 proj_w, proj_b):
    bf = _bf()
    sh = {}
    lnw = np.stack([
        np.broadcast_to(ln1_w, (128, D_MODEL)),
        np.broadcast_to(ln1_b, (128, D_MODEL)),
        np.broadcast_to(ln2_w, (128, D_MODEL)),
        np.broadcast_to(ln2_b, (128, D_MODEL))], axis=1)
    sh["lnw"] = np.ascontiguousarray(lnw, np.float32)

    wt = in_proj_w.T.astype(np.float32)          # [1024, 4256]
    wt_pad = np.zeros((D_MODEL, 34 * 128), np.float32)
    wt_pad[:, :D_IN_PROJ] = wt
    w4 = wt_pad.reshape(KD, 128, 34, 128)        # [k, dp, e, ec]
    sh["wxbc"] = np.ascontiguousarray(
        w4[:, :, 16:34, :].transpose(1, 0, 2, 3)).astype(bf)
    sh["wz"] = np.ascontiguousarray(
        w4[:, :, 0:16, :].transpose(1, 0, 2, 3)).astype(bf)

    wo = out_proj_w.T.astype(np.float32)         # [2048, 1024]
    sh["wout"] = np.ascontiguousarray(
        wo.reshape(16, 128, DT, 128).transpose(1, 0, 2, 3)).astype(bf)
    wf = fc_w.T.astype(np.float32)               # [1024, 4096]
    sh["wfc"] = np.ascontiguousarray(
        wf.reshape(KD, 128, FT, 128).transpose(1, 0, 2, 3)).astype(bf)
    wp = proj_w.T.astype(np.float32)             # [4096, 1024]
    sh["wpj"] = np.ascontiguousarray(
        wp.reshape(FT, 128, DT, 128).transpose(1, 0, 2, 3)).astype(bf)

    cwp = np.zeros((CT * 128, D_CONV), np.float32)
    cwp[:CONV_DIM] = conv_w
    sh["convw"] = np.ascontiguousarray(
        cwp.reshape(CT, 128, D_CONV).transpose(1, 0, 2))
    cbp = np.zeros((CT * 128,), np.float32)
    cbp[:CONV_DIM] = conv_b
    sh["convb"] = np.ascontiguousarray(cbp.reshape(CT, 128).T)

    sh["dtb"] = dt_bias.reshape(32, 1).astype(np.float32)
    sh["negA"] = (-np.exp(A_log)).reshape(32, 1).astype(np.float32)
    sh["dcol"] = np.ascontiguousarray(
        np.repeat(D_param, HEADDIM).reshape(16, 128).T).astype(np.float32)
    sh["nwcol"] = np.ascontiguousarray(
        norm_w.reshape(16, 128).T).astype(np.float32)
    sh["fcb"] = np.ascontiguousarray(
        fc_b.reshape(FT, 128).T).astype(np.float32)
    sh["pjb"] = np.ascontiguousarray(
        proj_b.reshape(DT, 128).T).astype(np.float32)
    sh["d32"] = np.eye(32, dtype=np.float32).astype(bf)
    sh["idrep"] = np.ascontiguousarray(np.broadcast_to(
        np.eye(128, dtype=np.float32)[:, None, :], (128, 16, 128))).astype(bf)
    ident = np.eye(128, dtype=np.float32)
    sh["identb"] = ident.astype(bf)
    sh["identf"] = ident.astype(np.float32)
    return sh


def make_core_inputs(x, inputs_shared, tpc):
    """x: [TOK, D_MODEL] flattened full input."""
    maps = []
    for c in range(NCORES):
        m = dict(inputs_shared)
        s = c * tpc
        m["xin"] = np.ascontiguousarray(x[s:s + tpc], np.float32)
        if c % 2 == 1:
            m["xhalo"] = np.ascontiguousarray(x[s - 3:s], np.float32)
        else:
            m["xhalo"] = np.zeros((3, D_MODEL), np.float32)
        cm = np.zeros((128, 2), np.float32)
        cm[:, 0] = 1.0 if c % 2 == 0 else 0.0
        cm[:, 1] = 1.0 if c % 2 == 1 else 0.0
        m["cmask"] = cm
        maps.append(m)
    return maps


def _run(inputs, trace=False):
    from concourse.bass_utils import run_bass_kernel_spmd
    x = np.asarray(inputs["x"], np.float32)
    B, L, D = x.shape
    tpc = B * L // NCORES
    xf = x.reshape(B * L, D)
    args = {k: np.asarray(v, np.float32)
            for k, v in inputs.items() if k != "x"}
    sh = _prep_shared(**args)
    maps = make_core_inputs(xf, sh, tpc)
    if "nc" not in _CACHE:
        _CACHE["nc"] = build(tpc, sim=False)
    res = run_bass_kernel_spmd(_CACHE["nc"], maps,
                               core_ids=list(range(NCORES)), trace=trace)
    out = np.concatenate([res.results[c]["out"] for c in range(NCORES)],
                         axis=0)
    return out.reshape(B, L, D).astype(np.float32), res


def kernel(x, ln1_w, ln1_b, ln2_w, ln2_b, in_proj_w, conv_w, conv_b, dt_bias,
           A_log, D_param, norm_w, out_proj_w, fc_w, fc_b, proj_w, proj_b):
    out, _ = _run(dict(
        x=x, ln1_w=ln1_w, ln1_b=ln1_b, ln2_w=ln2_w, ln2_b=ln2_b,
        in_proj_w=in_proj_w, conv_w=conv_w, conv_b=conv_b, dt_bias=dt_bias,
        A_log=A_log, D_param=D_param, norm_w=norm_w, out_proj_w=out_proj_w,
        fc_w=fc_w, fc_b=fc_b, proj_w=proj_w, proj_b=proj_b))
    return out
